# revision 1
# baseline (speedup 1.0000x reference)
"""BevPoolV2 Trainium2 kernel (8-core SPMD).

Math (reference): for each point k in [0, K):
    vals[k, :] = feat[nhw(indices[k]), :] * depth.flat[indices[k]]
segment m (fixed length 5: points [5m, 5m+5)) sums vals rows, and the segment
sum is scatter-added into BEV row intervals[m, 2] of a (N*OH*OW, C) grid.

Distribution: segments are sharded by BEV-rank range across 8 cores (each core
owns 49152 output rows, split into two 24576-row halves so scatter indices fit
int16).  Each core:
  - dma_gather (SWDGE) pulls its points' feat rows from HBM into SBUF in a
    partition-per-segment layout ([128 segs, 5 pts, 128 ch] per block),
  - DVE multiplies by per-point depth values (host-gathered input, broadcast
    along channels via a stride-0 AP) and window-5-reduces to segment sums,
  - dma_scatter_add (SWDGE CCE-add) accumulates segment rows into the output
    half in HBM.  Scatters are serialized; the host assigns duplicate ranks to
    distinct chunks so no single scatter call carries a duplicate row.
Host-side work is index manipulation / sharding: computing feat-row ids and
per-point depth values, permuting points into the chunk layout, and packing
the wrapped int16 index lists the SWDGE instructions expect.
"""
import sys

sys.path.insert(0, "/opt/trn_rl_repo")

import numpy as np

import concourse.bacc as bacc
import concourse.mybir as mybir
from concourse.bass_utils import run_bass_kernel_spmd
from concourse.library_config import mlp

# problem dims (hardcoded per contract)
N, D, H, W, C = 6, 118, 32, 88, 128
OH = OW = 256
K = 1_000_000
M = 200_000
HWp = H * W          # 2816
NFEAT = N * HWp      # 16896 feat rows
ROWS = N * OH * OW   # 393216 output rows

NCORES = 8
CROWS = ROWS // NCORES   # 49152 rows per core
HROWS = CROWS // 2       # 24576 rows per half (int16-safe)
TRASH = HROWS            # extra scratch row absorbing padded-slot zeros

P = 128
NCHUNK = 16              # chunks per core; even chunks -> half 0, odd -> half 1
SEG_PP = 14              # segment slots per partition per chunk
CHUNK_SEGS = P * SEG_PP  # 1792
JPP = SEG_PP * 5         # 70 points per partition per chunk
CHUNK_PTS = CHUNK_SEGS * 5  # 8960
NHALF_CHUNKS = NCHUNK // 2

_CACHED = {}


def _build_program():
    nc = bacc.Bacc("TRN2", target_bir_lowering=False, debug=False)
    feat = nc.dram_tensor("feat", [NFEAT, C], mybir.dt.float32, kind="ExternalInput")
    fidx = nc.dram_tensor("fidx", [P, NCHUNK * CHUNK_PTS // 16], mybir.dt.int16,
                          kind="ExternalInput")
    dval = nc.dram_tensor("dval", [P, NCHUNK * JPP], mybir.dt.float32,
                          kind="ExternalInput")
    sidx = nc.dram_tensor("sidx", [P, NCHUNK * CHUNK_SEGS // 16], mybir.dt.int16,
                          kind="ExternalInput")
    outs = [nc.dram_tensor(f"out{h}", [HROWS + 1, C], mybir.dt.float32,
                           kind="ExternalOutput") for h in range(2)]

    from contextlib import ExitStack
    NB = 2  # pipeline buffers
    with ExitStack() as st:
        fidx_sb = st.enter_context(
            nc.sbuf_tensor("fidx_sb", [P, NCHUNK * CHUNK_PTS // 16], mybir.dt.int16))
        dval_sb = st.enter_context(
            nc.sbuf_tensor("dval_sb", [P, NCHUNK * JPP], mybir.dt.float32))
        sidx_sb = st.enter_context(
            nc.sbuf_tensor("sidx_sb", [P, NCHUNK * CHUNK_SEGS // 16], mybir.dt.int16))
        vals = [st.enter_context(
            nc.sbuf_tensor(f"vals{i}", [P, JPP, C], mybir.dt.float32))
            for i in range(NB)]
        # all chunks' segment sums stay resident so the scatter phase runs
        # after every gather finished (gather/scatter DMA interleaving on the
        # SDMA engines measured ~50x slower than phase-separated)
        segs = st.enter_context(
            nc.sbuf_tensor("segs", [P, NCHUNK, SEG_PP, C], mybir.dt.float32))

        ld = st.enter_context(nc.semaphore("ld"))
        gsem = st.enter_context(nc.semaphore("gsem"))
        vsem = st.enter_context(nc.semaphore("vsem"))
        ssem = st.enter_context(nc.semaphore("ssem"))

        # index/weight loads on HWDGE (sync engine)
        nc.sync.dma_start(fidx_sb[:], fidx[:]).then_inc(ld, 16)
        nc.sync.dma_start(dval_sb[:], dval[:]).then_inc(ld, 16)
        nc.sync.dma_start(sidx_sb[:], sidx[:]).then_inc(ld, 16)

        nc.gpsimd.load_library(mlp)
        nc.gpsimd.wait_ge(ld, 48)

        FW = CHUNK_PTS // 16   # fidx words per chunk (per partition)
        SW = CHUNK_SEGS // 16  # sidx words per chunk

        # phase 1: gathers (POOL) pipelined with multiply+reduce (DVE)
        for c in range(NCHUNK):
            if c >= NB:
                nc.gpsimd.wait_ge(vsem, c - NB + 1)  # vals[c%NB] free
            nc.gpsimd.dma_gather(
                vals[c % NB][:], feat[:],
                fidx_sb[:, c * FW:(c + 1) * FW],
                CHUNK_PTS, CHUNK_PTS, C,
                single_packet=False,
            ).then_inc(gsem, 16)

            nc.vector.wait_ge(gsem, 16 * (c + 1))
            nc.vector.tensor_tensor(
                out=vals[c % NB][:],
                in0=vals[c % NB][:],
                in1=dval_sb[:, c * JPP:(c + 1) * JPP]
                    .unsqueeze(-1).to_broadcast([P, JPP, C]),
                op=mybir.AluOpType.mult,
            )
            nc.vector.tensor_reduce(
                out=segs[:, c],
                in_=vals[c % NB][:].rearrange("p (s f) c -> p s c f", f=5),
                axis=mybir.AxisListType.X,
                op=mybir.AluOpType.add,
            ).then_inc(vsem, 1)

        # phase 2: scatter-adds. Duplicate rows are always same-half (same
        # output tensor) and the host puts them in distinct chunks; same-half
        # calls are c and c+2, so scatter c only needs c-2 complete. Waiting
        # on ssem >= 16*(c-1) keeps the two halves' calls overlapped while
        # still ordering every same-tensor pair.
        nc.gpsimd.wait_ge(vsem, NCHUNK)
        for c in range(NCHUNK):
            if c >= 2:
                nc.gpsimd.wait_ge(ssem, 16 * (c - 1))
            nc.gpsimd.dma_scatter_add(
                outs[c % 2][:], segs[:, c],
                sidx_sb[:, c * SW:(c + 1) * SW],
                CHUNK_SEGS, CHUNK_SEGS, C,
                single_packet=False,
            ).then_inc(ssem, 16)

        nc.gpsimd.wait_ge(ssem, 16 * NCHUNK)
    nc.compile()
    return nc


def _wrap16(lists):
    """[..., n] index list -> SWDGE wrapped layout [..., 128, n//16]:
    list[q] lands at partition q%16 word q//16, replicated across the 8
    16-partition groups."""
    *lead, n = lists.shape
    w = lists.reshape(*lead, n // 16, 16)
    w = np.moveaxis(w, -1, -2)            # [..., 16, n//16]
    return np.broadcast_to(
        w[..., None, :, :], (*lead, 8, 16, n // 16)
    ).reshape(*lead, 128, n // 16)


def _prep_core(m_sel, hrow, half, fidx_pts, dval_pts):
    """Assign core's segments (global ids m_sel) to NCHUNK x CHUNK_SEGS slots
    with no duplicate output row within any single chunk, then build the
    gather/value/scatter arrays in device layout."""
    nseg = m_sel.size
    # order by (half, row) to find duplicate groups
    order = np.lexsort((hrow, half))
    m_s, hrow_s, half_s = m_sel[order], hrow[order], half[order]
    newgrp = np.ones(nseg, dtype=bool)
    newgrp[1:] = (hrow_s[1:] != hrow_s[:-1]) | (half_s[1:] != half_s[:-1])
    gstart = np.flatnonzero(newgrp)
    occ = np.arange(nseg) - np.repeat(gstart, np.diff(np.append(gstart, nseg)))

    # chunk ids for half h are h, h+2, ..., h+14 (c % 2 == h)
    fills = np.zeros(NCHUNK, np.int64)
    chunk_of = np.empty(nseg, np.int64)
    # uniques (occ==0): round-robin within the half for balance
    for h in (0, 1):
        u = np.flatnonzero((half_s == h) & (occ == 0))
        cids = h + 2 * (np.arange(u.size) % NHALF_CHUNKS)
        chunk_of[u] = cids
        np.add.at(fills, cids, 1)
    # duplicates: greedy, avoiding chunks already used by the same row
    dups = np.flatnonzero(occ > 0)
    for i in dups:
        h = half_s[i]
        used = set(chunk_of[i - occ[i]:i])  # group members precede i in order
        cand = [h + 2 * j for j in range(NHALF_CHUNKS)]
        cand = [cc for cc in cand if cc not in used]
        assert cand, "output row duplicated more than NCHUNK/2 times"
        cc = min(cand, key=lambda x: fills[x])
        chunk_of[i] = cc
        fills[cc] += 1
    assert fills.max() <= CHUNK_SEGS, f"chunk overflow: {fills.max()}"

    # slot assignment: fill each chunk sequentially
    slot = np.empty(nseg, np.int64)
    corder = np.argsort(chunk_of, kind="stable")
    cnt = np.zeros(NCHUNK, np.int64)
    sorted_chunks = chunk_of[corder]
    # per-chunk running index (vectorized cumcount)
    cc_change = np.ones(nseg, dtype=bool)
    cc_change[1:] = sorted_chunks[1:] != sorted_chunks[:-1]
    cstart = np.flatnonzero(cc_change)
    slot[corder] = np.arange(nseg) - np.repeat(
        cstart, np.diff(np.append(cstart, nseg)))

    # build padded per-(chunk, slot) segment grid
    m_grid = np.full((NCHUNK, CHUNK_SEGS), -1, np.int64)
    row_grid = np.full((NCHUNK, CHUNK_SEGS), TRASH, np.int64)
    m_grid[chunk_of, slot] = m_s
    row_grid[chunk_of, slot] = hrow_s

    # slot t -> partition p = t % 128, seg-slot s = t // 128
    pts = 5 * m_grid[:, :, None] + np.arange(5)           # [NC, CS, 5]
    validm = (m_grid >= 0)[:, :, None]
    f_v = np.where(validm, fidx_pts[np.clip(pts, 0, K - 1)], 0).astype(np.int16)
    d_v = np.where(validm, dval_pts[np.clip(pts, 0, K - 1)], 0.0).astype(np.float32)
    # [NC, CS, 5] -> [NC, s(14), p(128), i(5)]
    f_v = f_v.reshape(NCHUNK, SEG_PP, P, 5)
    d_v = d_v.reshape(NCHUNK, SEG_PP, P, 5)
    # gather list position q = j2*128 + p with j2 = s*5 + i
    fidx_list = f_v.transpose(0, 1, 3, 2).reshape(NCHUNK, CHUNK_PTS)
    # dval device layout [128, NCHUNK*JPP]: dval[p, c*JPP + j2]
    dval_dev = d_v.transpose(2, 0, 1, 3).reshape(P, NCHUNK * JPP)
    # scatter list position q2 = s*128 + p = t
    sidx_list = row_grid.astype(np.int16)                  # [NC, CS]

    fidx_dev = _wrap16(fidx_list).transpose(1, 0, 2).reshape(P, -1)
    sidx_dev = _wrap16(sidx_list).transpose(1, 0, 2).reshape(P, -1)
    return (np.ascontiguousarray(fidx_dev), np.ascontiguousarray(dval_dev),
            np.ascontiguousarray(sidx_dev))


def prepare_in_maps(feat, depth, indices, intervals):
    """Host-side sharding: full inputs -> per-core input dicts."""
    idx = indices.astype(np.int64)
    fidx_pts = (idx // (D * HWp) * HWp + idx % HWp).astype(np.int16)
    dval_pts = np.ascontiguousarray(depth).reshape(-1)[idx].astype(np.float32)

    iv = np.asarray(intervals, dtype=np.int64)
    assert np.array_equal(iv[:, 0], 5 * np.arange(M)), "expected fixed-5 segments"
    assert np.array_equal(iv[:, 1], iv[:, 0] + 5), "expected fixed-5 segments"
    ranks = iv[:, 2]
    core = ranks // CROWS
    locrow = ranks % CROWS
    half = locrow // HROWS
    hrow = locrow % HROWS

    featf = np.ascontiguousarray(feat, dtype=np.float32).reshape(NFEAT, C)
    in_maps = []
    for c in range(NCORES):
        sel = np.flatnonzero(core == c)
        assert sel.size <= NCHUNK * CHUNK_SEGS, "core segment overflow"
        fidx_dev, dval_dev, sidx_dev = _prep_core(
            sel, hrow[sel], half[sel], fidx_pts, dval_pts)
        in_maps.append({
            "feat": featf,
            "fidx": fidx_dev,
            "dval": dval_dev,
            "sidx": sidx_dev,
        })
    return in_maps


def assemble_output(results):
    parts = []
    for c in range(NCORES):
        parts.append(results[c]["out0"][:HROWS])
        parts.append(results[c]["out1"][:HROWS])
    out_flat = np.concatenate(parts, axis=0)      # [ROWS, C]
    return out_flat.reshape(N, OH, OW, C).transpose(0, 3, 1, 2)


def get_program():
    if "nc" not in _CACHED:
        _CACHED["nc"] = _build_program()
    return _CACHED["nc"]


def kernel(feat, depth, indices, intervals):
    nc = get_program()
    in_maps = prepare_in_maps(np.asarray(feat), np.asarray(depth),
                              np.asarray(indices), np.asarray(intervals))
    res = run_bass_kernel_spmd(nc, in_maps, core_ids=list(range(NCORES)))
    return assemble_output(res.results)

